# revision 26
# baseline (speedup 1.0000x reference)
"""Trainium2 Bass kernel for DiffusionPriorNetwork (dense transformer).

Sharding: data-parallel over batch (32 seqs/core on 8 cores), no collectives.
On-chip layout is feature-major ([feature_partition, token]).

v3 changes over v2 (the 11.3ms baseline) -> 8.38ms measured:
  * Attention restructured for tensor-engine continuity (v2 left the PE
    idle ~50% of the attention phase):
      - per-seq softmax software-pipelined at depth 2: AV(s) issues two
        iterations after scores(s), so the scalar exp + DVE bias-mul
        chain is hidden; the AV psum drains to SBUF immediately so its
        bank frees at DVE speed (psum is the pipeline-depth limiter:
        sc 2x2 + ot 2 + rb 2 = 8 banks).
      - K/V staged once per layer: kkT2 [128, 32, 81] f16 with null-kv in
        column 0 (k duplicated in both partition halves via the [wk|wk]
        stationary), V transposed per seq into vTt [81, 32, 65] with an
        appended ones column (denominator comes free from the AV matmul).
      - attention output written as aoT [p=par*64+d, hh, t] (par1 via one
        small DMA scatter per seq); Wout rows are host-side permuted to
        match (inner_idx), so Wout runs as plain fp8 DoubleRow matmuls.
      - merged norm ops (one tsq mul, one xn mul with a stride-0
        broadcast rbx AP), batched V-transpose drains, drains split
        across scalar/vector engines - the attention phase is otherwise
        instruction-COUNT bound on those engines.
      - the whole layer's rmsnorm/xn8 is hoisted ahead of the pass loop
        (8 chunks pipelining among themselves), so the q/kv projections
        never stall on the norm chain at pass boundaries (-0.76ms).
  * Last layer computes only the learned-query token (the only output):
    q/scores/AV/Wout/FFN run on 32 tokens instead of 2560.
  * FFN unchanged from v2 (f16, tensor-saturated at 99% busy; fp8 fails
    the error budget - raw fp8 FFN measures 5e-2 rel err vs the 2e-2
    gate, and error-compensated fp8 is slower than f16 since DoubleRow
    is 1 cycle/row on this hardware, not the cost model's 0.5).

  Hardware pitfalls found on the way (documented for future sessions):
  * matmul psum outputs at a partition offset (tile_position col 64)
    raise NRT_EXEC_UNIT_UNRECOVERABLE - never write psum rows 64-127
    from a matmul whose lhsT free size is 64.
  * psum tiles smaller than a full 2KB bank crash when the allocator
    packs several accumulation regions into one bank - always pad psum
    tiles to 512 f32 per partition.
  * matmul moving operands need a unit-stride innermost free dim.
"""
import math
import os
import sys

import numpy as np

sys.path.insert(0, '/opt/trn_rl_repo')

import json

import ml_dtypes
import concourse.bass as bass
import concourse.mybir as mybir
import concourse.bass_utils as _bass_utils
import concourse.bass2jax as _bass2jax
from concourse.masks import make_identity
from concourse.tile import TileContext
from concourse.bass_utils import run_bass_kernel_spmd


def _split_multi_waits(bir: bytes) -> bytes:
    """The installed walrus accepts one sync-wait per instruction; hoist
    extra waits onto EventSemaphore nops inserted just before, on the same
    engine (identical blocking semantics)."""
    obj = json.loads(bir)
    ctr = 0
    changed = False
    for fn in obj.get("functions", []):
        for bb in fn.get("blocks", []):
            out = []
            for ins in bb.get("instructions", []):
                si = ins.get("sync_info")
                waits = (si or {}).get("on_wait") or []
                if len(waits) > 1 and ins.get("engine"):
                    for w in waits[:-1]:
                        ctr += 1
                        out.append({
                            "debug": ins.get("debug", 0),
                            "engine": ins["engine"],
                            "ins": [], "outs": [],
                            "name": f"waitnop-{ctr}",
                            "opcode": "EventSemaphore",
                            "sync_info": {"on_update": [], "on_wait": [w]},
                        })
                    si["on_wait"] = [waits[-1]]
                    changed = True
                out.append(ins)
            bb["instructions"] = out
    if not changed:
        return bir
    return json.dumps(obj).encode()


_orig_compile_bir_kernel = _bass_utils.compile_bir_kernel


def _patched_compile_bir_kernel(bir_json, tmpdir, neff_name="file.neff"):
    if isinstance(bir_json, str):
        bir_json = bir_json.encode()
    return _orig_compile_bir_kernel(_split_multi_waits(bir_json), tmpdir,
                                    neff_name=neff_name)


_bass_utils.compile_bir_kernel = _patched_compile_bir_kernel
_bass2jax.compile_bir_kernel = _patched_compile_bir_kernel

B, L, DIM, DEPTH, HEADS, DH = 256, 77, 768, 12, 12, 64
TSTEPS, BUCKETS, MAXDIST = 1000, 32, 128
EPS = 1e-5
NSEQ = 80
NKEY = 81
FF = 4 * DIM          # 3072
KT = DIM // 128       # 6
NPAIR = DIM // 256    # 3 (fp8 DoubleRow pairs over DIM)
FKT = FF // 128       # 24
NCORES = 8
BLOC = B // NCORES    # 32
TLOC = BLOC * NSEQ    # 2560
NPASS = 4
PSEQ = BLOC // NPASS  # 8 seqs per attention pass
PTOK = PSEQ * NSEQ    # 640
CH = 512              # ffn token chunk
NCH = TLOC // CH      # 5

F32 = mybir.dt.float32
F16 = mybir.dt.float16
F8 = mybir.dt.float8e4
AF = mybir.ActivationFunctionType
ALU = mybir.AluOpType
DRM = mybir.MatmulPerfMode.DoubleRow
NEG = -30000.0

WS = 64.0                     # fp8 weight scale (exact power of 2)
QDS = 1.0 / (WS * DH ** 0.5)  # q descale, includes DH^-0.5
KDS = 1.0 / WS                # k/v descale
LNB = 0.5 * math.log(DIM)     # inv = exp(-0.5*ln(sumsq) + LNB)

_DEPTH = int(os.environ.get('KERNEL_DEPTH', DEPTH))
_STAGE = int(os.environ.get('KERNEL_STAGE', '99'))


def _host_bias(table):
    """rel_pos_bias(NSEQ, NKEY) ported from the reference; [HEADS, 80, 81]."""
    q = np.arange(NSEQ)
    k = np.arange(NKEY)
    rel = k[None, :] - q[:, None]
    n = np.maximum(-rel, 0)
    max_exact = BUCKETS // 2
    is_small = n < max_exact
    nf = np.maximum(n, 1).astype(np.float32)
    val_large = max_exact + (
        np.log(nf / max_exact) / math.log(MAXDIST / max_exact) * (BUCKETS - max_exact)
    ).astype(np.int32)
    val_large = np.minimum(val_large, BUCKETS - 1)
    bucket = np.where(is_small, n, val_large)
    return np.transpose(table[bucket], (2, 0, 1)).astype(np.float32)


class _G:
    """Per-build handles shared between helpers."""
    pass


def _norm_chunk(nc, g, np_, nps, t0, n, out8, out_sl, fp8):
    """rmsnorm factor for tokens [t0, t0+n); writes xn (fp8 or f16) into
    out8[:, :, out_sl]."""
    sq = nps.tile([1, 512], F32, tag="sq")
    tsq = np_.tile([128, KT, 512], F16, tag="tsq")
    nc.vector.tensor_mul(tsq[:, :, :n], g.xT[:, :, t0:t0 + n],
                         g.xT[:, :, t0:t0 + n])
    for kt in range(KT):
        nc.tensor.matmul(sq[:, :n], g.ones16[:], tsq[:, kt, :n],
                         start=(kt == 0), stop=(kt == KT - 1))
    lnv = np_.tile([1, 512], F32, tag="lnv")
    nc.scalar.activation(lnv[:, :n], sq[:, :n], AF.Ln, bias=g.eps_ap[:1])
    inv = np_.tile([1, 512], F16, tag="inv")
    nc.scalar.activation(inv[:, :n], lnv[:, :n], AF.Exp,
                         bias=g.lnb_ap[:1], scale=-0.5)
    rbx = nps.tile([128, 512], F32, tag="rbx")
    nc.tensor.matmul(rbx[:, :n], g.onesrow[:], inv[:, :n], start=True, stop=True)
    rbb = rbx[:, :n].rearrange("p (o x) -> p o x", o=1).to_broadcast([128, KT, n])
    nc.vector.tensor_mul(out8[:, :, out_sl], g.xT[:, :, t0:t0 + n], rbb)


def _attention(nc, tc, g, lyr, last):
    """One attention layer over all 32 seqs (4 passes of 8)."""
    # null-kv columns (column 0 of every 81-key group)
    nc.vector.tensor_copy(g.kkT2[:, :, 0], g.nk2[:].to_broadcast([128, BLOC]))
    nc.vector.tensor_copy(g.vTg2[:, :, 0], g.nv[:].to_broadcast([DH, BLOC]))
    nc.vector.tensor_copy(g.vTt[:, :, DH],
                          g.ones32[:NKEY].to_broadcast([NKEY, BLOC]))

    # whole-layer rmsnorm + xn8 upfront (chunks pipeline among themselves,
    # so projections below never stall on the norm chain mid-pass)
    with tc.tile_pool(name="anrm", bufs=3) as np_, \
         tc.tile_pool(name="anps", bufs=3, space="PSUM") as nps:
        for c8 in range(2 * NPASS):
            _norm_chunk(nc, g, np_, nps, c8 * 320, 320, g.xn8,
                        slice(c8 * 320, c8 * 320 + 320), True)
    if last:
        # stage the query-token columns (local col 79 of each seq)
        nc.vector.tensor_copy(
            g.xn8q[:],
            g.xn8.rearrange("p k (s i) -> p k s i", i=NSEQ)[:, :, :, NSEQ - 1])

    for p in range(NPASS):
        p0 = p * PTOK
        with tc.tile_pool(name="aps", bufs=4, space="PSUM") as aps, \
             tc.tile_pool(name="trp", bufs=2, space="PSUM") as trp:
            # q projection -> qT f16 [128, 6, PTOK]
            if not last:
                for m in range(KT):
                    for h in range(2):
                        hsl = slice(h * 320, h * 320 + 320)
                        gsl = slice(p0 + h * 320, p0 + h * 320 + 320)
                        qps = aps.tile([128, 320], F32, tag="p")
                        for j in range(NPAIR):
                            nc.tensor.matmul(qps[:], g.wq[:, j, :, m * 128:(m + 1) * 128],
                                             g.xn8[:, 2 * j:2 * j + 2, gsl],
                                             start=(j == 0), stop=(j == NPAIR - 1),
                                             perf_mode=DRM)
                        if (m + h) % 2 == 0:
                            nc.scalar.mul(g.qT[:, m, hsl], qps[:], QDS)
                        else:
                            nc.vector.tensor_scalar_mul(g.qT[:, m, hsl],
                                                        qps[:], QDS)
            # k/v projection + drains into kkT2 / vTg2
            for h in range(2 if _STAGE >= 2 else 0):
                gsl = slice(p0 + h * 320, p0 + h * 320 + 320)
                s0 = p * PSEQ + h * 4
                kps = aps.tile([128, 320], F32, tag="p")
                for j in range(NPAIR):
                    nc.tensor.matmul(kps[:], g.wkk[:, j], g.xn8[:, 2 * j:2 * j + 2, gsl],
                                     start=(j == 0), stop=(j == NPAIR - 1),
                                     perf_mode=DRM)
                nc.scalar.mul(g.kkT2[:, s0:s0 + 4, 1:],
                              kps.rearrange("p (s i) -> p s i", s=4), KDS)
                vps = aps.tile([128, 320], F32, tag="p")
                for j in range(NPAIR):
                    nc.tensor.matmul(vps[:DH, :], g.wv[:, j], g.xn8[:, 2 * j:2 * j + 2, gsl],
                                     start=(j == 0), stop=(j == NPAIR - 1),
                                     perf_mode=DRM)
                nc.scalar.mul(g.vTg2[:, s0:s0 + 4, 1:],
                              vps[:DH].rearrange("p (s i) -> p s i", s=4), KDS)
            # per-seq V transpose -> vTt [81, s, 64]; copies batched
            if _STAGE >= 3:
                tr = trp.tile([128, PSEQ * DH // 2], F32, tag="tr")
                trv = tr.bitcast(F16)
                for i in range(PSEQ):
                    s = p * PSEQ + i
                    nc.tensor.transpose(trv[:NKEY, i * DH:(i + 1) * DH],
                                        g.vTg2[:, s, :], g.id16[:DH, :DH])
                nc.vector.tensor_copy(
                    g.vTt[:, p * PSEQ:(p + 1) * PSEQ, :DH],
                    trv[:NKEY, :].rearrange("p (s d) -> p s d", s=PSEQ))

        if last:
            continue

        # ---- scores / softmax / AV, software-pipelined over the 8 seqs ----
        # depth-2 skew: AV(s) issues two iterations after scores(s), so the
        # scalar exp + DVE bias-mul chain is fully hidden.  ot drains to SBUF
        # immediately so the AV psum bank frees at DVE speed.
        with tc.tile_pool(name="scp", bufs=2, space="PSUM") as scp, \
             tc.tile_pool(name="otp", bufs=1, space="PSUM") as otp, \
             tc.tile_pool(name="rdp", bufs=1, space="PSUM") as rdp, \
             tc.tile_pool(name="etp", bufs=4) as etp:
            ring = {}
            ring2 = {}
            for i in range(PSEQ + 3):
                if i < PSEQ:
                    s = p * PSEQ + i
                    isl = slice(i * NSEQ, (i + 1) * NSEQ)
                    # 512-padded par stride keeps each half psum-bank aligned
                    sc = scp.tile([NKEY, 2, 512], F32, tag="sc")
                    for par in range(2):
                        psl = slice(par * 64, par * 64 + 64)
                        nc.tensor.matmul(sc[:, par, :480], g.kkT2[psl, s, :],
                                         g.qT[psl, :, isl], start=True, stop=True)
                    et0 = etp.tile([NKEY, 2, 480], F16, tag="et0")
                    nc.scalar.activation(et0[:], sc[:, :, :480], AF.Exp,
                                         bias=g.maskT[:, s:s + 1])
                    et = etp.tile([NKEY, 2, 480], F16, tag="et")
                    nc.vector.tensor_mul(et[:], et0[:], g.expB3[:])
                    ring[i] = (s, et)
                if 2 <= i <= PSEQ + 1:
                    s, et1 = ring[i - 2]
                    # AV with appended ones column -> denominators in row 64
                    ot = otp.tile([DH + 1, 2, 512], F32, tag="ot")
                    for par in range(2):
                        nc.tensor.matmul(ot[:, par, :480], g.vTt[:, s, :],
                                         et1[:, par, :], start=True, stop=True)
                    otsb = etp.tile([DH + 1, 2, 480], F16, tag="otsb")
                    nc.vector.tensor_copy(otsb[:], ot[:, :, :480])
                    lnd = etp.tile([1, 2, 480], F16, tag="lnd")
                    nc.scalar.activation(lnd[:], otsb[DH:DH + 1, :, :], AF.Ln)
                    rec = etp.tile([1, 2, 480], F16, tag="rec")
                    nc.scalar.activation(rec[:], lnd[:], AF.Exp, scale=-1.0)
                    rb = rdp.tile([DH, 2, 512], F32, tag="rd")
                    for par in range(2):
                        nc.tensor.matmul(rb[:, par, :480], g.onesrow[:1, :DH],
                                         rec[:, par, :], start=True, stop=True)
                    rb16 = etp.tile([DH, 2, 480], F16, tag="rb16")
                    nc.vector.tensor_copy(rb16[:], rb[:, :, :480])
                    ring2[i - 2] = (s, otsb, rb16)
                if 3 <= i:
                    s, ot2, rb2 = ring2[i - 3]
                    iloc = s - p * PSEQ
                    isl2 = slice(iloc * NSEQ, (iloc + 1) * NSEQ)
                    tmp1 = etp.tile([DH, 2, KT, NSEQ], F8, tag="tmp1")
                    nc.vector.tensor_mul(
                        tmp1[:],
                        ot2[:DH, :, :].rearrange("p b (h i) -> p b h i", h=KT),
                        rb2[:].rearrange("p b (h i) -> p b h i", h=KT))
                    nc.sync.dma_start(g.aoT[:DH, :, isl2], tmp1[:, 0])
                    nc.sync.dma_start(g.aoT[DH:128, :, isl2], tmp1[:, 1])

        # ---- Wout + residual ----
        with tc.tile_pool(name="wps", bufs=4, space="PSUM") as wps:
            for m in range(KT):
                for h in range(2):
                    hsl = slice(h * 320, h * 320 + 320)
                    ops_ = wps.tile([128, 320], F32, tag="p")
                    for j in range(NPAIR):
                        nc.tensor.matmul(ops_[:], g.wo[:, j, :, m * 128:(m + 1) * 128],
                                         g.aoT[:, 2 * j:2 * j + 2, hsl],
                                         start=(j == 0), stop=(j == NPAIR - 1),
                                         perf_mode=DRM)
                    nc.vector.scalar_tensor_tensor(
                        g.xT[:, m, p0 + h * 320:p0 + h * 320 + 320], ops_[:],
                        1.0 / WS, g.xT[:, m, p0 + h * 320:p0 + h * 320 + 320],
                        op0=ALU.mult, op1=ALU.add)

    if not last:
        return

    # ---- last layer: queries are only the learned-query token ----
    with tc.tile_pool(name="lqp", bufs=2, space="PSUM") as aps:
        for m in range(KT if _STAGE >= 4 else 0):
            qps = aps.tile([128, 512], F32, tag="p")
            for j in range(NPAIR):
                nc.tensor.matmul(qps[:, :BLOC], g.wq[:, j, :, m * 128:(m + 1) * 128],
                                 g.xn8q[:, 2 * j:2 * j + 2, :],
                                 start=(j == 0), stop=(j == NPAIR - 1),
                                 perf_mode=DRM)
            nc.scalar.mul(g.qTmini[:, m, :], qps[:, :BLOC], QDS)
        if _STAGE >= 4:
            nc.vector.tensor_copy(g.qTl[:],
                                  g.qTmini.rearrange("p m s -> p s m"))

    with tc.tile_pool(name="scpl", bufs=2, space="PSUM") as scp, \
         tc.tile_pool(name="otpl", bufs=1, space="PSUM") as otp, \
         tc.tile_pool(name="rdpl", bufs=1, space="PSUM") as rdp, \
         tc.tile_pool(name="etpl", bufs=3) as etp:
        ring = {}
        ring2 = {}
        for i in range(BLOC + 2 if _STAGE >= 5 else 0):
            if i < BLOC:
                s = i
                sc = scp.tile([NKEY, 2, 512], F32, tag="sc")
                for par in range(2):
                    psl = slice(par * 64, par * 64 + 64)
                    nc.tensor.matmul(sc[:, par, :KT], g.kkT2[psl, s, :],
                                     g.qTl[psl, s, :], start=True, stop=True)
                et0 = etp.tile([NKEY, 2, KT], F16, tag="et0")
                nc.scalar.activation(et0[:], sc[:, :, :KT], AF.Exp,
                                     bias=g.maskT[:, s:s + 1])
                et = etp.tile([NKEY, 2, KT], F16, tag="et")
                nc.vector.tensor_mul(et[:], et0[:], g.expBL[:])
                ring[i] = (s, et)
            if 1 <= i <= BLOC:
                s, et1 = ring[i - 1]
                ot = otp.tile([DH + 1, 2, 512], F32, tag="ot")
                for par in range(2):
                    nc.tensor.matmul(ot[:, par, :KT], g.vTt[:, s, :],
                                     et1[:, par, :], start=True, stop=True)
                lnd = etp.tile([1, 2, KT], F16, tag="lnd")
                nc.scalar.activation(lnd[:], ot[DH:DH + 1, :, :KT], AF.Ln)
                rec = etp.tile([1, 2, KT], F16, tag="rec")
                nc.scalar.activation(rec[:], lnd[:], AF.Exp, scale=-1.0)
                rb = rdp.tile([DH, 2, 512], F32, tag="rd")
                for par in range(2):
                    nc.tensor.matmul(rb[:, par, :KT], g.onesrow[:1, :DH],
                                     rec[:, par, :], start=True, stop=True)
                rb16 = etp.tile([DH, 2, KT], F16, tag="rb16")
                nc.vector.tensor_copy(rb16[:], rb[:, :, :KT])
                ring2[i - 1] = (s, ot, rb16)
            if 2 <= i:
                s, ot2, rb2 = ring2[i - 2]
                for par in range(2):
                    nc.vector.tensor_mul(g.ots8[:, par, s % PSEQ, :],
                                         ot2[:DH, par, :KT], rb2[:, par, :])
                if s % PSEQ == PSEQ - 1:
                    sl8 = slice(s - PSEQ + 1, s + 1)
                    nc.vector.tensor_copy(
                        g.aoTl[:DH, :, sl8],
                        g.ots8[:, 0].rearrange("p s h -> p h s"))
                    nc.vector.tensor_copy(
                        g.tmp8l[:, :, sl8],
                        g.ots8[:, 1].rearrange("p s h -> p h s"))
        nc.sync.dma_start(g.aoTl[DH:128, :, :], g.tmp8l[:])

    with tc.tile_pool(name="wpsl", bufs=2, space="PSUM") as wps:
        for m in range(KT if _STAGE >= 6 else 0):
            ops_ = wps.tile([128, 512], F32, tag="p")
            for j in range(NPAIR):
                nc.tensor.matmul(ops_[:, :BLOC], g.wo[:, j, :, m * 128:(m + 1) * 128],
                                 g.aoTl[:, 2 * j:2 * j + 2, :],
                                 start=(j == 0), stop=(j == NPAIR - 1),
                                 perf_mode=DRM)
            nc.vector.scalar_tensor_tensor(
                g.xTl[:, m, :], ops_[:, :BLOC], 1.0 / WS, g.xTl[:, m, :],
                op0=ALU.mult, op1=ALU.add)


def _norm_pass(nc, tc, g, inv):
    """inv[0, t] = sqrt(DIM)/sqrt(sum_f x[f,t]^2 + EPS) for all tokens."""
    with tc.tile_pool(name="nrm", bufs=2) as np_, \
         tc.tile_pool(name="nrm_ps", bufs=2, space="PSUM") as nps:
        for c in range(NCH):
            sl = slice(c * CH, (c + 1) * CH)
            sq = nps.tile([1, CH], F32, tag="sq")
            for kt in range(KT):
                tsq = np_.tile([128, CH], F16, tag="tsq")
                nc.vector.tensor_mul(tsq[:], g.xT[:, kt, sl], g.xT[:, kt, sl])
                nc.tensor.matmul(sq[:], g.ones16[:], tsq[:],
                                 start=(kt == 0), stop=(kt == KT - 1))
            lnv = np_.tile([1, CH], F32, tag="lnv")
            nc.scalar.activation(lnv[:], sq[:], AF.Ln, bias=g.eps_ap[:1])
            nc.scalar.activation(inv[:, sl], lnv[:], AF.Exp,
                                 bias=g.lnb_ap[:1], scale=-0.5)


def _ffn_full(nc, tc, g, lyr, w1_d, w2_d):
    """f16 FFN over all 2560 tokens (baseline structure)."""
    with tc.tile_pool(name="ffn", bufs=1) as fp, \
         tc.tile_pool(name="ffw", bufs=4) as fwp, \
         tc.tile_pool(name="ffw2", bufs=3) as fw2, \
         tc.tile_pool(name="ffb", bufs=2) as fb:
        inv2 = fp.tile([1, TLOC], F16, tag="inv2")
        _norm_pass(nc, tc, g, inv2)

        with tc.tile_pool(name="fps", bufs=1, space="PSUM") as fps, \
             tc.tile_pool(name="wps", bufs=1, space="PSUM") as wps:
            # chunk pairs: each W1/W2 stationary streams both chunks
            # back-to-back (identical consecutive weight loads)
            for cset in ((0, 1), (2, 3), (4,)):
                xns, ffts, sls = [], [], []
                for c in cset:
                    ci = len(sls)
                    sl = slice(c * CH, (c + 1) * CH)
                    rbx = fps.tile([128, CH], F32, tag=f"a{ci}",
                                   name=f"rbx{ci}")
                    nc.tensor.matmul(rbx[:], g.onesrow[:], inv2[:, sl],
                                     start=True, stop=True)
                    xn = fb.tile([128, KT, CH], F16, tag="xn2")
                    rbb = rbx[:].rearrange("p (o x) -> p o x", o=1) \
                        .to_broadcast([128, KT, CH])
                    nc.vector.tensor_mul(xn[:], g.xT[:, :, sl], rbb)
                    xns.append(xn)
                    ffT = fp.tile([128, FKT, CH], F16, tag=f"ffT{ci}",
                                  name=f"ffT{ci}")
                    ffts.append(ffT)
                    sls.append(sl)
                for mp in range(FKT):
                    w1b = fwp.tile([128, 2, KT, 128], F16, tag="w1b")
                    nc.sync.dma_start(w1b[:], w1_d[lyr, :, mp])
                    pa = [fps.tile([128, CH], F32, tag=f"a{ci}",
                                   name=f"pa{ci}") for ci in range(len(cset))]
                    pg = [fps.tile([128, CH], F32, tag=f"g{ci}",
                                   name=f"pg{ci}") for ci in range(len(cset))]
                    for ag, ps in ((0, pa), (1, pg)):
                        for kt in range(KT):
                            for ci in range(len(cset)):
                                nc.tensor.matmul(ps[ci][:], w1b[:, ag, kt],
                                                 xns[ci][:, kt, :],
                                                 start=(kt == 0),
                                                 stop=(kt == KT - 1))
                    for ci in range(len(cset)):
                        sil = fb.tile([128, CH], F16, tag="sil")
                        nc.scalar.activation(sil[:], pg[ci][:], AF.Silu)
                        nc.vector.tensor_mul(ffts[ci][:, mp, :], pa[ci][:], sil[:])
                for m in range(KT):
                    w2b = fw2.tile([128, FKT, 128], F16, tag="w2b")
                    nc.sync.dma_start(w2b[:], w2_d[lyr, :, m])
                    py = [wps.tile([128, CH], F32, tag=f"y{ci}",
                                   name=f"py{ci}") for ci in range(len(cset))]
                    for fk in range(FKT):
                        for ci in range(len(cset)):
                            nc.tensor.matmul(py[ci][:], w2b[:, fk],
                                             ffts[ci][:, fk, :],
                                             start=(fk == 0),
                                             stop=(fk == FKT - 1))
                    for ci in range(len(cset)):
                        nc.vector.tensor_add(g.xT[:, m, sls[ci]], py[ci][:],
                                             g.xT[:, m, sls[ci]])


def _ffn_last(nc, tc, g, lyr, w1_d, w2_d):
    """FFN on the 32 query tokens only (strided view of xT)."""
    n = BLOC
    with tc.tile_pool(name="lfw", bufs=4) as fwp, \
         tc.tile_pool(name="lfw2", bufs=3) as fw2, \
         tc.tile_pool(name="lfb", bufs=2) as fb, \
         tc.tile_pool(name="lfs", bufs=1) as fs:
        xn = fs.tile([128, KT, n], F16, tag="xn2")
        ffT = fs.tile([128, FKT, n], F16, tag="ffT")
        with tc.tile_pool(name="lnps", bufs=1, space="PSUM") as nps:
            sq = nps.tile([1, 512], F32, tag="sq")
            for kt in range(KT):
                tsq = fb.tile([128, n], F16, tag="tsq")
                nc.vector.tensor_mul(tsq[:], g.xTl[:, kt, :], g.xTl[:, kt, :])
                nc.tensor.matmul(sq[:, :n], g.ones16[:], tsq[:],
                                 start=(kt == 0), stop=(kt == KT - 1))
            lnv = fb.tile([1, n], F32, tag="lnv")
            nc.scalar.activation(lnv[:], sq[:, :n], AF.Ln, bias=g.eps_ap[:1])
            inv = fb.tile([1, n], F16, tag="inv")
            nc.scalar.activation(inv[:], lnv[:], AF.Exp, bias=g.lnb_ap[:1],
                                 scale=-0.5)
            rbx = nps.tile([128, 512], F32, tag="rbx")
            nc.tensor.matmul(rbx[:, :n], g.onesrow[:], inv[:], start=True, stop=True)
            for kt in range(KT):
                nc.vector.tensor_mul(xn[:, kt, :], g.xTl[:, kt, :], rbx[:, :n])
        with tc.tile_pool(name="lfps", bufs=2, space="PSUM") as fps, \
             tc.tile_pool(name="lwps", bufs=2, space="PSUM") as wps:
            for mp in range(FKT):
                w1b = fwp.tile([128, 2, KT, 128], F16, tag="w1b")
                nc.sync.dma_start(w1b[:], w1_d[lyr, :, mp])
                a_ps = fps.tile([128, 512], F32, tag="a")
                g_ps = fps.tile([128, 512], F32, tag="g")
                for kt in range(KT):
                    nc.tensor.matmul(a_ps[:, :n], w1b[:, 0, kt], xn[:, kt, :],
                                     start=(kt == 0), stop=(kt == KT - 1))
                for kt in range(KT):
                    nc.tensor.matmul(g_ps[:, :n], w1b[:, 1, kt], xn[:, kt, :],
                                     start=(kt == 0), stop=(kt == KT - 1))
                sil = fb.tile([128, n], F16, tag="sil")
                nc.scalar.activation(sil[:], g_ps[:, :n], AF.Silu)
                nc.vector.tensor_mul(ffT[:, mp, :], a_ps[:, :n], sil[:])
            for m in range(KT):
                w2b = fw2.tile([128, FKT, 128], F16, tag="w2b")
                nc.sync.dma_start(w2b[:], w2_d[lyr, :, m])
                ops_ = wps.tile([128, 512], F32, tag="w2o")
                for fk in range(FKT):
                    nc.tensor.matmul(ops_[:, :n], w2b[:, fk], ffT[:, fk, :],
                                     start=(fk == 0), stop=(fk == KT * 4 - 1))
                nc.vector.tensor_add(g.xTl[:, m, :], ops_[:, :n], g.xTl[:, m, :])


_BUILD_CACHE = {}


def _build(depth):
    if depth in _BUILD_CACHE:
        return _BUILD_CACHE[depth]
    nc = bass.Bass()

    xT_d = nc.dram_tensor("xT", [128, KT, TLOC], F16, kind="ExternalInput")
    wq_d = nc.dram_tensor("wq", [depth, 128, NPAIR, 2, DIM], F8, kind="ExternalInput")
    wkk_d = nc.dram_tensor("wkk", [depth, 128, NPAIR, 2, 128], F8, kind="ExternalInput")
    wv_d = nc.dram_tensor("wv", [depth, 128, NPAIR, 2, DH], F8, kind="ExternalInput")
    wo_d = nc.dram_tensor("wo", [depth, 128, NPAIR, 2, DIM], F8, kind="ExternalInput")
    w1_d = nc.dram_tensor("w1", [depth, 128, FKT, 2, KT, 128], F16,
                          kind="ExternalInput")
    w2_d = nc.dram_tensor("w2", [depth, 128, KT, FKT, 128], F16, kind="ExternalInput")
    nk2_d = nc.dram_tensor("nk2", [depth, 128, 1], F32, kind="ExternalInput")
    nv_d = nc.dram_tensor("nv", [depth, DH, 1], F32, kind="ExternalInput")
    expB_d = nc.dram_tensor("expB", [NKEY, 960], F16, kind="ExternalInput")
    mask_d = nc.dram_tensor("maskT", [NKEY, BLOC], F32, kind="ExternalInput")
    out_d = nc.dram_tensor("out", [128, KT, BLOC], F16, kind="ExternalOutput")

    with TileContext(nc) as tc:
        with nc.allow_low_precision(reason="fp8 matmuls / f16 softmax by design"), \
             tc.tile_pool(name="persist", bufs=1) as pp:
            g = _G()
            g.xT = pp.tile([128, KT, TLOC], F16)
            nc.sync.dma_start(g.xT[:], xT_d[:])
            g.xTl = g.xT.rearrange("p k (s i) -> p k s i", i=NSEQ)[:, :, :, NSEQ - 1]
            g.expB = pp.tile([NKEY, 960], F16)
            nc.sync.dma_start(g.expB[:], expB_d[:])
            g.expB3 = g.expB.rearrange("p (b x) -> p b x", b=2)
            g.maskT = pp.tile([NKEY, BLOC], F32)
            nc.sync.dma_start(g.maskT[:], mask_d[:])
            ident = pp.tile([128, 128], F32)
            make_identity(nc, ident)
            g.id16 = pp.tile([128, 128], F16)
            nc.vector.tensor_copy(g.id16[:], ident[:])
            g.ones32 = pp.tile([128, 1], F32)
            nc.vector.memset(g.ones32[:], 1.0)
            g.ones16 = pp.tile([128, 1], F16)
            nc.vector.tensor_copy(g.ones16[:], g.ones32[:])
            g.onesrow = pp.tile([1, 128], F16)
            nc.vector.memset(g.onesrow[:], 1.0)
            g.eps_ap = pp.tile([128, 1], F32)
            nc.vector.memset(g.eps_ap[:], EPS)
            g.lnb_ap = pp.tile([128, 1], F32)
            nc.vector.memset(g.lnb_ap[:], LNB)

            # per-layer staged tensors
            g.kkT2 = pp.tile([128, BLOC, NKEY], F16)
            g.vTg2 = pp.tile([DH, BLOC, NKEY], F16)
            g.vTt = pp.tile([NKEY, BLOC, DH + 1], F16)
            g.qT = pp.tile([128, KT, PTOK], F16)
            g.xn8 = pp.tile([128, KT, TLOC], F8)
            g.aoT = pp.tile([128, KT, PTOK], F8)
            g.xn8q = pp.tile([128, KT, BLOC], F8)
            g.qTl = pp.tile([128, BLOC, KT], F16)
            g.qTmini = pp.tile([128, KT, BLOC], F16)
            g.ots8 = pp.tile([DH, 2, PSEQ, KT], F16)
            g.expBL = pp.tile([NKEY, 2, KT], F16)
            nc.vector.tensor_copy(
                g.expBL[:],
                g.expB3.rearrange("p b (h i) -> p b h i", i=NSEQ)[:, :, :, NSEQ - 1])
            g.aoTl = pp.tile([128, KT, BLOC], F8)
            g.tmp8l = pp.tile([DH, KT, BLOC], F8)
            # attention weights (re-DMA'd per layer)
            g.wq = pp.tile([128, NPAIR, 2, DIM], F8)
            g.wkk = pp.tile([128, NPAIR, 2, 128], F8)
            g.wv = pp.tile([128, NPAIR, 2, DH], F8)
            g.wo = pp.tile([128, NPAIR, 2, DIM], F8)
            g.nk2 = pp.tile([128, 1], F16)
            g.nv = pp.tile([DH, 1], F16)

            for lyr in range(depth):
                last = (lyr == depth - 1)
                nc.sync.dma_start(g.wq[:], wq_d[lyr])
                nc.sync.dma_start(g.wkk[:], wkk_d[lyr])
                nc.sync.dma_start(g.wv[:], wv_d[lyr])
                nc.sync.dma_start(g.wo[:], wo_d[lyr])
                nk2f = pp.tile([128, 1], F32, tag=f"nk2f")
                nc.sync.dma_start(nk2f[:], nk2_d[lyr])
                nc.vector.tensor_copy(g.nk2[:], nk2f[:])
                nvf = pp.tile([DH, 1], F32, tag=f"nvf")
                nc.sync.dma_start(nvf[:], nv_d[lyr])
                nc.vector.tensor_copy(g.nv[:], nvf[:])
                _attention(nc, tc, g, lyr, last)
                if last:
                    if _STAGE >= 7:
                        _ffn_last(nc, tc, g, lyr, w1_d, w2_d)
                else:
                    _ffn_full(nc, tc, g, lyr, w1_d, w2_d)

            outT = pp.tile([128, KT, BLOC], F16)
            nc.vector.tensor_copy(outT[:], g.xTl[:])
            nc.sync.dma_start(out_d[:], outT[:])

    _BUILD_CACHE[depth] = nc
    return nc


def _to8(w):
    return np.clip(w * WS, -240.0, 240.0).astype(ml_dtypes.float8_e4m3)


def kernel(**inputs):
    depth = _DEPTH
    te = np.asarray(inputs['text_encodings'], np.float32)
    tex = np.asarray(inputs['text_embed'], np.float32)
    tt = np.asarray(inputs['time_emb_table'], np.float32)
    lq = np.asarray(inputs['learned_query'], np.float32)
    rbt = np.asarray(inputs['rel_bias_table'], np.float32)
    ag = np.asarray(inputs['attn_gamma'], np.float32)
    Wq = np.asarray(inputs['Wq'], np.float32)
    Wkv = np.asarray(inputs['Wkv'], np.float32)
    Wout = np.asarray(inputs['Wout'], np.float32)
    nkv = np.asarray(inputs['null_kv'], np.float32)
    fg = np.asarray(inputs['ff_gamma'], np.float32)
    W1 = np.asarray(inputs['Wff1'], np.float32)
    W2 = np.asarray(inputs['Wff2'], np.float32)
    ts = np.asarray(inputs['diffusion_timesteps'])
    mask = np.asarray(inputs['mask'])

    time_embed = tt[ts]
    tokens = np.concatenate(
        [te, tex[:, None, :], time_embed[:, None, :],
         np.broadcast_to(lq, (B, 1, DIM))], axis=1).astype(np.float32)

    # gamma folds into the norm-consuming weights; the rmsnorm sqrt(DIM)
    # factor lives in `inv` on-chip; DH^-0.5 lives in the q descale.
    wq_eff = ag[:, :, None] * Wq
    wkv_eff = ag[:, :, None] * Wkv
    wkk_eff = np.concatenate([wkv_eff[:, :, :DH], wkv_eff[:, :, :DH]], axis=2)
    wv_eff = wkv_eff[:, :, DH:]
    w1_eff = fg[:, :, None] * W1

    def pack8(w):
        # [depth, DIM, N] -> [depth, 128, 3, 2, N] fp8 (DoubleRow pairs)
        d, K, N = w.shape
        return np.ascontiguousarray(
            _to8(w).reshape(d, NPAIR, 2, 128, N).transpose(0, 3, 1, 2, 4))

    # Wout rows permuted to the AV-output layout: aoT[p, hh, t] holds head
    # h = 2*hh + (p>=64), dim d = p%64  ->  Wout row (2*(2j+s)+(p>=64))*64+p%64
    pidx = np.arange(128)
    par = (pidx >= 64).astype(np.int64)
    dd = pidx % 64
    inner_idx = np.empty((128, NPAIR, 2), np.int64)
    for j in range(NPAIR):
        for s in range(2):
            inner_idx[:, j, s] = (2 * (2 * j + s) + par) * 64 + dd
    wo8 = _to8(Wout[:depth])           # [d, 768, 768] fp8
    woP = np.ascontiguousarray(wo8[:, inner_idx, :])  # [d, 128, 3, 2, 768]

    # scores multiplicative bias exp(relpos + causal): [81, 2(par), 6(hh), 80(i)]
    bias = _host_bias(rbt)
    causal = (np.arange(NKEY)[None, :] > np.arange(NSEQ)[:, None] + 1)
    bias = bias + np.where(causal, NEG, 0.0)[None]
    bt = np.zeros((NKEY, 2, KT, NSEQ), np.float32)
    for h in range(HEADS):
        bt[:, h % 2, h // 2, :] = bias[h].T
    with np.errstate(under='ignore'):
        expB = np.ascontiguousarray(np.exp(bt.reshape(NKEY, 960))).astype(np.float16)

    # per-batch additive key-mask rows [B, 81] (applied inside exp)
    m = np.zeros((B, NKEY), np.float32)
    not_all = mask.any(axis=-1)
    m[:, 1:L + 1] = np.where(mask, 0.0, NEG)
    m[:, L + 1] = np.where(not_all, 0.0, NEG)

    w1f = w1_eff[:depth].astype(np.float16)  # [d, DIM, 2*FF]
    d = w1f.shape[0]
    # [d, kt, p, ag, mp, n] -> [d, 128(p), 24(mp), 2(ag), 6(kt), 128(n)]
    w1p = np.ascontiguousarray(
        w1f.reshape(d, KT, 128, 2, FKT, 128).transpose(0, 2, 4, 3, 1, 5))
    # [d, FF, DIM] -> [d, 128(p), 6(m), 24(fk), 128(n)]
    w2p = np.ascontiguousarray(
        W2[:depth].astype(np.float16).reshape(d, FKT, 128, KT, 128)
        .transpose(0, 2, 3, 1, 4))

    nc = _build(depth)
    shared = {
        "wq": pack8(wq_eff[:depth]),
        "wkk": pack8(wkk_eff[:depth]),
        "wv": pack8(wv_eff[:depth]),
        "wo": woP,
        "w1": w1p,
        "w2": w2p,
        "nk2": np.ascontiguousarray(
            np.concatenate([nkv[:depth, 0], nkv[:depth, 0]], axis=1)
            .reshape(depth, 128, 1)),
        "nv": np.ascontiguousarray(nkv[:depth, 1].reshape(depth, DH, 1)),
        "expB": expB,
    }
    in_maps = []
    for c in range(NCORES):
        bsl = slice(c * BLOC, (c + 1) * BLOC)
        im = dict(shared)
        xTc = tokens[bsl].reshape(TLOC, DIM).T  # [DIM, TLOC]
        im["xT"] = np.ascontiguousarray(
            xTc.reshape(KT, 128, TLOC).transpose(1, 0, 2)).astype(np.float16)
        im["maskT"] = np.ascontiguousarray(m[bsl].T)
        in_maps.append(im)

    res = run_bass_kernel_spmd(nc, in_maps, core_ids=list(range(NCORES)),
                               trace=bool(int(os.environ.get('KERNEL_TRACE', '0'))))
    outs = []
    for c in range(NCORES):
        o = res.results[c]["out"]  # [128(p), KT, BLOC] f16
        outs.append(np.transpose(o, (2, 1, 0)).reshape(BLOC, DIM).astype(np.float32))
    kernel.last_results = res
    return np.concatenate(outs, axis=0)


# revision 30
# speedup vs baseline: 1.1928x; 1.1928x over previous
"""Trainium2 Bass kernel for DiffusionPriorNetwork (dense transformer).

Sharding: data-parallel over batch (32 seqs/core on 8 cores), no collectives.
On-chip layout is feature-major ([feature_partition, token]).

v3 changes over v2 (the 11.3ms baseline) -> 8.38ms measured:
  * Attention restructured for tensor-engine continuity (v2 left the PE
    idle ~50% of the attention phase):
      - per-seq softmax software-pipelined at depth 2: AV(s) issues two
        iterations after scores(s), so the scalar exp + DVE bias-mul
        chain is hidden; the AV psum drains to SBUF immediately so its
        bank frees at DVE speed (psum is the pipeline-depth limiter:
        sc 2x2 + ot 2 + rb 2 = 8 banks).
      - K/V staged once per layer: kkT2 [128, 32, 81] f16 with null-kv in
        column 0 (k duplicated in both partition halves via the [wk|wk]
        stationary), V transposed per seq into vTt [81, 32, 65] with an
        appended ones column (denominator comes free from the AV matmul).
      - attention output written as aoT [p=par*64+d, hh, t] (par1 via one
        small DMA scatter per seq); Wout rows are host-side permuted to
        match (inner_idx), so Wout runs as plain fp8 DoubleRow matmuls.
      - merged norm ops (one tsq mul, one xn mul with a stride-0
        broadcast rbx AP), batched V-transpose drains, drains split
        across scalar/vector engines - the attention phase is otherwise
        instruction-COUNT bound on those engines.
      - the whole layer's rmsnorm/xn8 is hoisted ahead of the pass loop
        (8 chunks pipelining among themselves), so the q/kv projections
        never stall on the norm chain at pass boundaries (-0.76ms).
  * Last layer computes only the learned-query token (the only output):
    q/scores/AV/Wout/FFN run on 32 tokens instead of 2560.
  * FFN unchanged from v2 (f16, tensor-saturated at 99% busy; fp8 fails
    the error budget - raw fp8 FFN measures 5e-2 rel err vs the 2e-2
    gate, and error-compensated fp8 is slower than f16 since DoubleRow
    is 1 cycle/row on this hardware, not the cost model's 0.5).

  Hardware pitfalls found on the way (documented for future sessions):
  * matmul psum outputs at a partition offset (tile_position col 64)
    raise NRT_EXEC_UNIT_UNRECOVERABLE - never write psum rows 64-127
    from a matmul whose lhsT free size is 64.
  * psum tiles smaller than a full 2KB bank crash when the allocator
    packs several accumulation regions into one bank - always pad psum
    tiles to 512 f32 per partition.
  * matmul moving operands need a unit-stride innermost free dim.
"""
import math
import os
import sys

import numpy as np

sys.path.insert(0, '/opt/trn_rl_repo')

import json

import ml_dtypes
import concourse.bass as bass
import concourse.mybir as mybir
import concourse.bass_utils as _bass_utils
import concourse.bass2jax as _bass2jax
from concourse.masks import make_identity
from concourse.tile import TileContext
from concourse.bass_utils import run_bass_kernel_spmd


def _split_multi_waits(bir: bytes) -> bytes:
    """The installed walrus accepts one sync-wait per instruction; hoist
    extra waits onto EventSemaphore nops inserted just before, on the same
    engine (identical blocking semantics)."""
    obj = json.loads(bir)
    ctr = 0
    changed = False
    for fn in obj.get("functions", []):
        for bb in fn.get("blocks", []):
            out = []
            for ins in bb.get("instructions", []):
                si = ins.get("sync_info")
                waits = (si or {}).get("on_wait") or []
                if len(waits) > 1 and ins.get("engine"):
                    for w in waits[:-1]:
                        ctr += 1
                        out.append({
                            "debug": ins.get("debug", 0),
                            "engine": ins["engine"],
                            "ins": [], "outs": [],
                            "name": f"waitnop-{ctr}",
                            "opcode": "EventSemaphore",
                            "sync_info": {"on_update": [], "on_wait": [w]},
                        })
                    si["on_wait"] = [waits[-1]]
                    changed = True
                out.append(ins)
            bb["instructions"] = out
    if not changed:
        return bir
    return json.dumps(obj).encode()


_orig_compile_bir_kernel = _bass_utils.compile_bir_kernel


def _patched_compile_bir_kernel(bir_json, tmpdir, neff_name="file.neff"):
    if isinstance(bir_json, str):
        bir_json = bir_json.encode()
    return _orig_compile_bir_kernel(_split_multi_waits(bir_json), tmpdir,
                                    neff_name=neff_name)


_bass_utils.compile_bir_kernel = _patched_compile_bir_kernel
_bass2jax.compile_bir_kernel = _patched_compile_bir_kernel

B, L, DIM, DEPTH, HEADS, DH = 256, 77, 768, 12, 12, 64
TSTEPS, BUCKETS, MAXDIST = 1000, 32, 128
EPS = 1e-5
NSEQ = 80
NKEY = 81
FF = 4 * DIM          # 3072
KT = DIM // 128       # 6
NPAIR = DIM // 256    # 3 (fp8 DoubleRow pairs over DIM)
FKT = FF // 128       # 24
NCORES = 8
BLOC = B // NCORES    # 32
TLOC = BLOC * NSEQ    # 2560
NPASS = 4
PSEQ = BLOC // NPASS  # 8 seqs per attention pass
PTOK = PSEQ * NSEQ    # 640
CH = 512              # ffn token chunk
NCH = TLOC // CH      # 5

F32 = mybir.dt.float32
F16 = mybir.dt.float16
F8 = mybir.dt.float8e4
AF = mybir.ActivationFunctionType
ALU = mybir.AluOpType
DRM = mybir.MatmulPerfMode.DoubleRow
NEG = -30000.0

WS = 64.0                     # fp8 weight scale (exact power of 2)
QDS = 1.0 / (WS * DH ** 0.5)  # q descale, includes DH^-0.5
KDS = 1.0 / WS                # k/v descale
LNB = 0.5 * math.log(DIM)     # inv = exp(-0.5*ln(sumsq) + LNB)

_DEPTH = int(os.environ.get('KERNEL_DEPTH', DEPTH))
_STAGE = int(os.environ.get('KERNEL_STAGE', '99'))


def _host_bias(table):
    """rel_pos_bias(NSEQ, NKEY) ported from the reference; [HEADS, 80, 81]."""
    q = np.arange(NSEQ)
    k = np.arange(NKEY)
    rel = k[None, :] - q[:, None]
    n = np.maximum(-rel, 0)
    max_exact = BUCKETS // 2
    is_small = n < max_exact
    nf = np.maximum(n, 1).astype(np.float32)
    val_large = max_exact + (
        np.log(nf / max_exact) / math.log(MAXDIST / max_exact) * (BUCKETS - max_exact)
    ).astype(np.int32)
    val_large = np.minimum(val_large, BUCKETS - 1)
    bucket = np.where(is_small, n, val_large)
    return np.transpose(table[bucket], (2, 0, 1)).astype(np.float32)


class _G:
    """Per-build handles shared between helpers."""
    pass


def _norm_chunk(nc, g, np_, nps, t0, n, out8, out_sl, fp8):
    """rmsnorm factor for tokens [t0, t0+n); writes xn (fp8 or f16) into
    out8[:, :, out_sl]."""
    sq = nps.tile([1, 512], F32, tag="sq")
    tsq = np_.tile([128, KT, 512], F16, tag="tsq")
    nc.vector.tensor_mul(tsq[:, :, :n], g.xT[:, :, t0:t0 + n],
                         g.xT[:, :, t0:t0 + n])
    for kt in range(KT):
        nc.tensor.matmul(sq[:, :n], g.ones16[:], tsq[:, kt, :n],
                         start=(kt == 0), stop=(kt == KT - 1))
    lnv = np_.tile([1, 512], F32, tag="lnv")
    nc.scalar.activation(lnv[:, :n], sq[:, :n], AF.Ln, bias=g.eps_ap[:1])
    inv = np_.tile([1, 512], F16, tag="inv")
    nc.scalar.activation(inv[:, :n], lnv[:, :n], AF.Exp,
                         bias=g.lnb_ap[:1], scale=-0.5)
    rbx = nps.tile([128, 512], F32, tag="rbx")
    nc.tensor.matmul(rbx[:, :n], g.onesrow[:], inv[:, :n], start=True, stop=True)
    rbb = rbx[:, :n].rearrange("p (o x) -> p o x", o=1).to_broadcast([128, KT, n])
    nc.vector.tensor_mul(out8[:, :, out_sl], g.xT[:, :, t0:t0 + n], rbb)


def _attention(nc, tc, g, lyr, last):
    """One attention layer over all 32 seqs (4 passes of 8)."""
    # null-kv columns (column 0 of every 81-key group)
    nc.vector.tensor_copy(g.kkT2[:, :, 0], g.nk2[:].to_broadcast([128, BLOC]))
    nc.vector.tensor_copy(g.vTg2[:, :, 0], g.nv[:].to_broadcast([DH, BLOC]))
    nc.vector.tensor_copy(g.vTt[:, :, DH],
                          g.ones32[:NKEY].to_broadcast([NKEY, BLOC]))

    # whole-layer rmsnorm + xn8 upfront (chunks pipeline among themselves,
    # so projections below never stall on the norm chain mid-pass)
    with tc.tile_pool(name="anrm", bufs=3) as np_, \
         tc.tile_pool(name="anps", bufs=3, space="PSUM") as nps:
        for c8 in range(2 * NPASS):
            _norm_chunk(nc, g, np_, nps, c8 * 320, 320, g.xn8,
                        slice(c8 * 320, c8 * 320 + 320), True)
    if last:
        # stage the query-token columns (local col 79 of each seq)
        nc.vector.tensor_copy(
            g.xn8q[:],
            g.xn8.rearrange("p k (s i) -> p k s i", i=NSEQ)[:, :, :, NSEQ - 1])

    for p in range(NPASS):
        p0 = p * PTOK
        with tc.tile_pool(name="aps", bufs=4, space="PSUM") as aps, \
             tc.tile_pool(name="trp", bufs=2, space="PSUM") as trp:
            # q projection -> qT f16 [128, 6, PTOK]
            if not last:
                for m in range(KT):
                    for h in range(2):
                        hsl = slice(h * 320, h * 320 + 320)
                        gsl = slice(p0 + h * 320, p0 + h * 320 + 320)
                        qps = aps.tile([128, 320], F32, tag="p")
                        for j in range(NPAIR):
                            nc.tensor.matmul(qps[:], g.wq[:, j, :, m * 128:(m + 1) * 128],
                                             g.xn8[:, 2 * j:2 * j + 2, gsl],
                                             start=(j == 0), stop=(j == NPAIR - 1),
                                             perf_mode=DRM)
                        if (m + h) % 2 == 0:
                            nc.scalar.mul(g.qT[:, m, hsl], qps[:], QDS)
                        else:
                            nc.vector.tensor_scalar_mul(g.qT[:, m, hsl],
                                                        qps[:], QDS)
            # k/v projection + drains into kkT2 / vTg2
            for h in range(2 if _STAGE >= 2 else 0):
                gsl = slice(p0 + h * 320, p0 + h * 320 + 320)
                s0 = p * PSEQ + h * 4
                kps = aps.tile([128, 320], F32, tag="p")
                for j in range(NPAIR):
                    nc.tensor.matmul(kps[:], g.wkk[:, j], g.xn8[:, 2 * j:2 * j + 2, gsl],
                                     start=(j == 0), stop=(j == NPAIR - 1),
                                     perf_mode=DRM)
                nc.scalar.mul(g.kkT2[:, s0:s0 + 4, 1:],
                              kps.rearrange("p (s i) -> p s i", s=4), KDS)
                vps = aps.tile([128, 320], F32, tag="p")
                for j in range(NPAIR):
                    nc.tensor.matmul(vps[:DH, :], g.wv[:, j], g.xn8[:, 2 * j:2 * j + 2, gsl],
                                     start=(j == 0), stop=(j == NPAIR - 1),
                                     perf_mode=DRM)
                nc.scalar.mul(g.vTg2[:, s0:s0 + 4, 1:],
                              vps[:DH].rearrange("p (s i) -> p s i", s=4), KDS)
            # per-seq V transpose -> vTt [81, s, 64]; copies batched
            if _STAGE >= 3:
                tr = trp.tile([128, PSEQ * DH // 2], F32, tag="tr")
                trv = tr.bitcast(F16)
                for i in range(PSEQ):
                    s = p * PSEQ + i
                    nc.tensor.transpose(trv[:NKEY, i * DH:(i + 1) * DH],
                                        g.vTg2[:, s, :], g.id16[:DH, :DH])
                nc.vector.tensor_copy(
                    g.vTt[:, p * PSEQ:(p + 1) * PSEQ, :DH],
                    trv[:NKEY, :].rearrange("p (s d) -> p s d", s=PSEQ))

        if last:
            continue

        # ---- scores / softmax / AV, software-pipelined over the 8 seqs ----
        # depth-2 skew: AV(s) issues two iterations after scores(s), so the
        # scalar exp + DVE bias-mul chain is fully hidden.  ot drains to SBUF
        # immediately so the AV psum bank frees at DVE speed.
        with tc.tile_pool(name="scp", bufs=2, space="PSUM") as scp, \
             tc.tile_pool(name="otp", bufs=1, space="PSUM") as otp, \
             tc.tile_pool(name="rdp", bufs=1, space="PSUM") as rdp, \
             tc.tile_pool(name="etp", bufs=6) as etp:
            ring = {}
            ring2 = {}
            for i in range(PSEQ + 3):
                if i < PSEQ:
                    s = p * PSEQ + i
                    isl = slice(i * NSEQ, (i + 1) * NSEQ)
                    # 512-padded par stride keeps each half psum-bank aligned
                    sc = scp.tile([NKEY, 2, 512], F32, tag="sc")
                    for par in range(2):
                        psl = slice(par * 64, par * 64 + 64)
                        nc.tensor.matmul(sc[:, par, :480], g.kkT2[psl, s, :],
                                         g.qT[psl, :, isl], start=True, stop=True)
                    et0 = etp.tile([NKEY, 2, 480], F16, tag="et0")
                    nc.scalar.activation(et0[:], sc[:, :, :480], AF.Exp,
                                         bias=g.maskT[:, s:s + 1])
                    et = etp.tile([NKEY, 2, 480], F16, tag="et")
                    nc.vector.tensor_mul(et[:], et0[:], g.expB3[:])
                    ring[i] = (s, et)
                if 2 <= i <= PSEQ + 1:
                    s, et1 = ring[i - 2]
                    # AV with appended ones column -> denominators in row 64
                    ot = otp.tile([DH + 1, 2, 512], F32, tag="ot")
                    for par in range(2):
                        nc.tensor.matmul(ot[:, par, :480], g.vTt[:, s, :],
                                         et1[:, par, :], start=True, stop=True)
                    otsb = etp.tile([DH + 1, 2, 480], F16, tag="otsb")
                    nc.vector.tensor_copy(otsb[:], ot[:, :, :480])
                    lnd = etp.tile([1, 2, 480], F16, tag="lnd")
                    nc.scalar.activation(lnd[:], otsb[DH:DH + 1, :, :], AF.Ln)
                    rec = etp.tile([1, 2, 480], F16, tag="rec")
                    nc.scalar.activation(rec[:], lnd[:], AF.Exp, scale=-1.0)
                    rb = rdp.tile([DH, 2, 512], F32, tag="rd")
                    for par in range(2):
                        nc.tensor.matmul(rb[:, par, :480], g.onesrow[:1, :DH],
                                         rec[:, par, :], start=True, stop=True)
                    rb16 = etp.tile([DH, 2, 480], F16, tag="rb16")
                    nc.vector.tensor_copy(rb16[:], rb[:, :, :480])
                    ring2[i - 2] = (s, otsb, rb16)
                if 3 <= i:
                    s, ot2, rb2 = ring2[i - 3]
                    iloc = s - p * PSEQ
                    isl2 = slice(iloc * NSEQ, (iloc + 1) * NSEQ)
                    tmp1 = etp.tile([DH, 2, KT, NSEQ], F8, tag="tmp1")
                    nc.vector.tensor_mul(
                        tmp1[:],
                        ot2[:DH, :, :].rearrange("p b (h i) -> p b h i", h=KT),
                        rb2[:].rearrange("p b (h i) -> p b h i", h=KT))
                    nc.sync.dma_start(g.aoT[:DH, :, isl2], tmp1[:, 0])
                    nc.sync.dma_start(g.aoT[DH:128, :, isl2], tmp1[:, 1])

        # ---- Wout + residual ----
        with tc.tile_pool(name="wps", bufs=4, space="PSUM") as wps:
            for m in range(KT):
                for h in range(2):
                    hsl = slice(h * 320, h * 320 + 320)
                    ops_ = wps.tile([128, 320], F32, tag="p")
                    for j in range(NPAIR):
                        nc.tensor.matmul(ops_[:], g.wo[:, j, :, m * 128:(m + 1) * 128],
                                         g.aoT[:, 2 * j:2 * j + 2, hsl],
                                         start=(j == 0), stop=(j == NPAIR - 1),
                                         perf_mode=DRM)
                    nc.vector.scalar_tensor_tensor(
                        g.xT[:, m, p0 + h * 320:p0 + h * 320 + 320], ops_[:],
                        1.0 / WS, g.xT[:, m, p0 + h * 320:p0 + h * 320 + 320],
                        op0=ALU.mult, op1=ALU.add)

    if not last:
        return

    # ---- last layer: queries are only the learned-query token ----
    with tc.tile_pool(name="lqp", bufs=2, space="PSUM") as aps:
        for m in range(KT if _STAGE >= 4 else 0):
            qps = aps.tile([128, 512], F32, tag="p")
            for j in range(NPAIR):
                nc.tensor.matmul(qps[:, :BLOC], g.wq[:, j, :, m * 128:(m + 1) * 128],
                                 g.xn8q[:, 2 * j:2 * j + 2, :],
                                 start=(j == 0), stop=(j == NPAIR - 1),
                                 perf_mode=DRM)
            nc.scalar.mul(g.qTmini[:, m, :], qps[:, :BLOC], QDS)
        if _STAGE >= 4:
            nc.vector.tensor_copy(g.qTl[:],
                                  g.qTmini.rearrange("p m s -> p s m"))

    with tc.tile_pool(name="scpl", bufs=2, space="PSUM") as scp, \
         tc.tile_pool(name="otpl", bufs=1, space="PSUM") as otp, \
         tc.tile_pool(name="rdpl", bufs=1, space="PSUM") as rdp, \
         tc.tile_pool(name="etpl", bufs=3) as etp:
        ring = {}
        ring2 = {}
        for i in range(BLOC + 2 if _STAGE >= 5 else 0):
            if i < BLOC:
                s = i
                sc = scp.tile([NKEY, 2, 512], F32, tag="sc")
                for par in range(2):
                    psl = slice(par * 64, par * 64 + 64)
                    nc.tensor.matmul(sc[:, par, :KT], g.kkT2[psl, s, :],
                                     g.qTl[psl, s, :], start=True, stop=True)
                et0 = etp.tile([NKEY, 2, KT], F16, tag="et0")
                nc.scalar.activation(et0[:], sc[:, :, :KT], AF.Exp,
                                     bias=g.maskT[:, s:s + 1])
                et = etp.tile([NKEY, 2, KT], F16, tag="et")
                nc.vector.tensor_mul(et[:], et0[:], g.expBL[:])
                ring[i] = (s, et)
            if 1 <= i <= BLOC:
                s, et1 = ring[i - 1]
                ot = otp.tile([DH + 1, 2, 512], F32, tag="ot")
                for par in range(2):
                    nc.tensor.matmul(ot[:, par, :KT], g.vTt[:, s, :],
                                     et1[:, par, :], start=True, stop=True)
                lnd = etp.tile([1, 2, KT], F16, tag="lnd")
                nc.scalar.activation(lnd[:], ot[DH:DH + 1, :, :KT], AF.Ln)
                rec = etp.tile([1, 2, KT], F16, tag="rec")
                nc.scalar.activation(rec[:], lnd[:], AF.Exp, scale=-1.0)
                rb = rdp.tile([DH, 2, 512], F32, tag="rd")
                for par in range(2):
                    nc.tensor.matmul(rb[:, par, :KT], g.onesrow[:1, :DH],
                                     rec[:, par, :], start=True, stop=True)
                rb16 = etp.tile([DH, 2, KT], F16, tag="rb16")
                nc.vector.tensor_copy(rb16[:], rb[:, :, :KT])
                ring2[i - 1] = (s, ot, rb16)
            if 2 <= i:
                s, ot2, rb2 = ring2[i - 2]
                for par in range(2):
                    nc.vector.tensor_mul(g.ots8[:, par, s % PSEQ, :],
                                         ot2[:DH, par, :KT], rb2[:, par, :])
                if s % PSEQ == PSEQ - 1:
                    sl8 = slice(s - PSEQ + 1, s + 1)
                    nc.vector.tensor_copy(
                        g.aoTl[:DH, :, sl8],
                        g.ots8[:, 0].rearrange("p s h -> p h s"))
                    nc.vector.tensor_copy(
                        g.tmp8l[:, :, sl8],
                        g.ots8[:, 1].rearrange("p s h -> p h s"))
        nc.sync.dma_start(g.aoTl[DH:128, :, :], g.tmp8l[:])

    with tc.tile_pool(name="wpsl", bufs=2, space="PSUM") as wps:
        for m in range(KT if _STAGE >= 6 else 0):
            ops_ = wps.tile([128, 512], F32, tag="p")
            for j in range(NPAIR):
                nc.tensor.matmul(ops_[:, :BLOC], g.wo[:, j, :, m * 128:(m + 1) * 128],
                                 g.aoTl[:, 2 * j:2 * j + 2, :],
                                 start=(j == 0), stop=(j == NPAIR - 1),
                                 perf_mode=DRM)
            nc.vector.scalar_tensor_tensor(
                g.xTl[:, m, :], ops_[:, :BLOC], 1.0 / WS, g.xTl[:, m, :],
                op0=ALU.mult, op1=ALU.add)


def _norm_pass(nc, tc, g, inv):
    """inv[0, t] = sqrt(DIM)/sqrt(sum_f x[f,t]^2 + EPS) for all tokens."""
    with tc.tile_pool(name="nrm", bufs=2) as np_, \
         tc.tile_pool(name="nrm_ps", bufs=2, space="PSUM") as nps:
        for c in range(NCH):
            sl = slice(c * CH, (c + 1) * CH)
            sq = nps.tile([1, CH], F32, tag="sq")
            for kt in range(KT):
                tsq = np_.tile([128, CH], F16, tag="tsq")
                nc.vector.tensor_mul(tsq[:], g.xT[:, kt, sl], g.xT[:, kt, sl])
                nc.tensor.matmul(sq[:], g.ones16[:], tsq[:],
                                 start=(kt == 0), stop=(kt == KT - 1))
            lnv = np_.tile([1, CH], F32, tag="lnv")
            nc.scalar.activation(lnv[:], sq[:], AF.Ln, bias=g.eps_ap[:1])
            nc.scalar.activation(inv[:, sl], lnv[:], AF.Exp,
                                 bias=g.lnb_ap[:1], scale=-0.5)


def _ffn_full(nc, tc, g, lyr, w1_d, w2_d):
    """f16 FFN over all 2560 tokens (baseline structure)."""
    with tc.tile_pool(name="ffn", bufs=1) as fp, \
         tc.tile_pool(name="ffw", bufs=4) as fwp, \
         tc.tile_pool(name="ffw2", bufs=3) as fw2, \
         tc.tile_pool(name="ffb", bufs=2) as fb:
        inv2 = fp.tile([1, TLOC], F16, tag="inv2")
        _norm_pass(nc, tc, g, inv2)

        with tc.tile_pool(name="fps", bufs=2, space="PSUM") as fps, \
             tc.tile_pool(name="wps", bufs=2, space="PSUM") as wps:
            for c in range(NCH):
                t0 = c * CH
                sl = slice(t0, t0 + CH)
                rbx = fps.tile([128, CH], F32, tag="a")
                nc.tensor.matmul(rbx[:], g.onesrow[:], inv2[:, sl],
                                 start=True, stop=True)
                xn = fb.tile([128, KT, CH], F16, tag="xn2")
                for kt in range(KT):
                    nc.vector.tensor_mul(xn[:, kt, :], g.xT[:, kt, sl], rbx[:])
                ffT = fp.tile([128, FKT, CH], F16, tag="ffT")
                for mp in range(FKT):
                    w1b = fwp.tile([128, 2, KT, 128], F16, tag="w1b")
                    nc.sync.dma_start(w1b[:], w1_d[lyr, :, mp])
                    a_ps = fps.tile([128, CH], F32, tag="a")
                    g_ps = fps.tile([128, CH], F32, tag="g")
                    for kt in range(KT):
                        nc.tensor.matmul(a_ps[:], w1b[:, 0, kt], xn[:, kt, :],
                                         start=(kt == 0), stop=(kt == KT - 1))
                    for kt in range(KT):
                        nc.tensor.matmul(g_ps[:], w1b[:, 1, kt], xn[:, kt, :],
                                         start=(kt == 0), stop=(kt == KT - 1))
                    sil = fb.tile([128, CH], F16, tag="sil")
                    nc.scalar.activation(sil[:], g_ps[:], AF.Silu)
                    nc.vector.tensor_mul(ffT[:, mp, :], a_ps[:], sil[:])
                for m in range(KT):
                    w2b = fw2.tile([128, FKT, 128], F16, tag="w2b")
                    nc.sync.dma_start(w2b[:], w2_d[lyr, :, m])
                    ops_ = wps.tile([128, CH], F32, tag="w2o")
                    for fk in range(FKT):
                        nc.tensor.matmul(ops_[:], w2b[:, fk], ffT[:, fk, :],
                                         start=(fk == 0), stop=(fk == FKT - 1))
                    nc.vector.tensor_add(g.xT[:, m, sl], ops_[:], g.xT[:, m, sl])


def _ffn_last(nc, tc, g, lyr, w1_d, w2_d):
    """FFN on the 32 query tokens only (strided view of xT)."""
    n = BLOC
    with tc.tile_pool(name="lfw", bufs=4) as fwp, \
         tc.tile_pool(name="lfw2", bufs=3) as fw2, \
         tc.tile_pool(name="lfb", bufs=2) as fb, \
         tc.tile_pool(name="lfs", bufs=1) as fs:
        xn = fs.tile([128, KT, n], F16, tag="xn2")
        ffT = fs.tile([128, FKT, n], F16, tag="ffT")
        with tc.tile_pool(name="lnps", bufs=1, space="PSUM") as nps:
            sq = nps.tile([1, 512], F32, tag="sq")
            for kt in range(KT):
                tsq = fb.tile([128, n], F16, tag="tsq")
                nc.vector.tensor_mul(tsq[:], g.xTl[:, kt, :], g.xTl[:, kt, :])
                nc.tensor.matmul(sq[:, :n], g.ones16[:], tsq[:],
                                 start=(kt == 0), stop=(kt == KT - 1))
            lnv = fb.tile([1, n], F32, tag="lnv")
            nc.scalar.activation(lnv[:], sq[:, :n], AF.Ln, bias=g.eps_ap[:1])
            inv = fb.tile([1, n], F16, tag="inv")
            nc.scalar.activation(inv[:], lnv[:], AF.Exp, bias=g.lnb_ap[:1],
                                 scale=-0.5)
            rbx = nps.tile([128, 512], F32, tag="rbx")
            nc.tensor.matmul(rbx[:, :n], g.onesrow[:], inv[:], start=True, stop=True)
            for kt in range(KT):
                nc.vector.tensor_mul(xn[:, kt, :], g.xTl[:, kt, :], rbx[:, :n])
        with tc.tile_pool(name="lfps", bufs=2, space="PSUM") as fps, \
             tc.tile_pool(name="lwps", bufs=2, space="PSUM") as wps:
            for mp in range(FKT):
                w1b = fwp.tile([128, 2, KT, 128], F16, tag="w1b")
                nc.sync.dma_start(w1b[:], w1_d[lyr, :, mp])
                a_ps = fps.tile([128, 512], F32, tag="a")
                g_ps = fps.tile([128, 512], F32, tag="g")
                for kt in range(KT):
                    nc.tensor.matmul(a_ps[:, :n], w1b[:, 0, kt], xn[:, kt, :],
                                     start=(kt == 0), stop=(kt == KT - 1))
                for kt in range(KT):
                    nc.tensor.matmul(g_ps[:, :n], w1b[:, 1, kt], xn[:, kt, :],
                                     start=(kt == 0), stop=(kt == KT - 1))
                sil = fb.tile([128, n], F16, tag="sil")
                nc.scalar.activation(sil[:], g_ps[:, :n], AF.Silu)
                nc.vector.tensor_mul(ffT[:, mp, :], a_ps[:, :n], sil[:])
            for m in range(KT):
                w2b = fw2.tile([128, FKT, 128], F16, tag="w2b")
                nc.sync.dma_start(w2b[:], w2_d[lyr, :, m])
                ops_ = wps.tile([128, 512], F32, tag="w2o")
                for fk in range(FKT):
                    nc.tensor.matmul(ops_[:, :n], w2b[:, fk], ffT[:, fk, :],
                                     start=(fk == 0), stop=(fk == KT * 4 - 1))
                nc.vector.tensor_add(g.xTl[:, m, :], ops_[:, :n], g.xTl[:, m, :])


_BUILD_CACHE = {}


def _build(depth):
    if depth in _BUILD_CACHE:
        return _BUILD_CACHE[depth]
    nc = bass.Bass()

    xT_d = nc.dram_tensor("xT", [128, KT, TLOC], F16, kind="ExternalInput")
    wq_d = nc.dram_tensor("wq", [depth, 128, NPAIR, 2, DIM], F8, kind="ExternalInput")
    wkk_d = nc.dram_tensor("wkk", [depth, 128, NPAIR, 2, 128], F8, kind="ExternalInput")
    wv_d = nc.dram_tensor("wv", [depth, 128, NPAIR, 2, DH], F8, kind="ExternalInput")
    wo_d = nc.dram_tensor("wo", [depth, 128, NPAIR, 2, DIM], F8, kind="ExternalInput")
    w1_d = nc.dram_tensor("w1", [depth, 128, FKT, 2, KT, 128], F16,
                          kind="ExternalInput")
    w2_d = nc.dram_tensor("w2", [depth, 128, KT, FKT, 128], F16, kind="ExternalInput")
    nk2_d = nc.dram_tensor("nk2", [depth, 128, 1], F32, kind="ExternalInput")
    nv_d = nc.dram_tensor("nv", [depth, DH, 1], F32, kind="ExternalInput")
    expB_d = nc.dram_tensor("expB", [NKEY, 960], F16, kind="ExternalInput")
    mask_d = nc.dram_tensor("maskT", [NKEY, BLOC], F32, kind="ExternalInput")
    out_d = nc.dram_tensor("out", [128, KT, BLOC], F16, kind="ExternalOutput")

    with TileContext(nc) as tc:
        with nc.allow_low_precision(reason="fp8 matmuls / f16 softmax by design"), \
             tc.tile_pool(name="persist", bufs=1) as pp:
            g = _G()
            g.xT = pp.tile([128, KT, TLOC], F16)
            nc.sync.dma_start(g.xT[:], xT_d[:])
            g.xTl = g.xT.rearrange("p k (s i) -> p k s i", i=NSEQ)[:, :, :, NSEQ - 1]
            g.expB = pp.tile([NKEY, 960], F16)
            nc.sync.dma_start(g.expB[:], expB_d[:])
            g.expB3 = g.expB.rearrange("p (b x) -> p b x", b=2)
            g.maskT = pp.tile([NKEY, BLOC], F32)
            nc.sync.dma_start(g.maskT[:], mask_d[:])
            ident = pp.tile([128, 128], F32)
            make_identity(nc, ident)
            g.id16 = pp.tile([128, 128], F16)
            nc.vector.tensor_copy(g.id16[:], ident[:])
            g.ones32 = pp.tile([128, 1], F32)
            nc.vector.memset(g.ones32[:], 1.0)
            g.ones16 = pp.tile([128, 1], F16)
            nc.vector.tensor_copy(g.ones16[:], g.ones32[:])
            g.onesrow = pp.tile([1, 128], F16)
            nc.vector.memset(g.onesrow[:], 1.0)
            g.eps_ap = pp.tile([128, 1], F32)
            nc.vector.memset(g.eps_ap[:], EPS)
            g.lnb_ap = pp.tile([128, 1], F32)
            nc.vector.memset(g.lnb_ap[:], LNB)

            # per-layer staged tensors
            g.kkT2 = pp.tile([128, BLOC, NKEY], F16)
            g.vTg2 = pp.tile([DH, BLOC, NKEY], F16)
            g.vTt = pp.tile([NKEY, BLOC, DH + 1], F16)
            g.qT = pp.tile([128, KT, PTOK], F16)
            g.xn8 = pp.tile([128, KT, TLOC], F8)
            g.aoT = pp.tile([128, KT, PTOK], F8)
            g.xn8q = pp.tile([128, KT, BLOC], F8)
            g.qTl = pp.tile([128, BLOC, KT], F16)
            g.qTmini = pp.tile([128, KT, BLOC], F16)
            g.ots8 = pp.tile([DH, 2, PSEQ, KT], F16)
            g.expBL = pp.tile([NKEY, 2, KT], F16)
            nc.vector.tensor_copy(
                g.expBL[:],
                g.expB3.rearrange("p b (h i) -> p b h i", i=NSEQ)[:, :, :, NSEQ - 1])
            g.aoTl = pp.tile([128, KT, BLOC], F8)
            g.tmp8l = pp.tile([DH, KT, BLOC], F8)
            # attention weights (re-DMA'd per layer)
            g.wq = pp.tile([128, NPAIR, 2, DIM], F8)
            g.wkk = pp.tile([128, NPAIR, 2, 128], F8)
            g.wv = pp.tile([128, NPAIR, 2, DH], F8)
            g.wo = pp.tile([128, NPAIR, 2, DIM], F8)
            g.nk2 = pp.tile([128, 1], F16)
            g.nv = pp.tile([DH, 1], F16)

            for lyr in range(depth):
                last = (lyr == depth - 1)
                nc.sync.dma_start(g.wq[:], wq_d[lyr])
                nc.sync.dma_start(g.wkk[:], wkk_d[lyr])
                nc.sync.dma_start(g.wv[:], wv_d[lyr])
                nc.sync.dma_start(g.wo[:], wo_d[lyr])
                nk2f = pp.tile([128, 1], F32, tag=f"nk2f")
                nc.sync.dma_start(nk2f[:], nk2_d[lyr])
                nc.vector.tensor_copy(g.nk2[:], nk2f[:])
                nvf = pp.tile([DH, 1], F32, tag=f"nvf")
                nc.sync.dma_start(nvf[:], nv_d[lyr])
                nc.vector.tensor_copy(g.nv[:], nvf[:])
                _attention(nc, tc, g, lyr, last)
                if last:
                    if _STAGE >= 7:
                        _ffn_last(nc, tc, g, lyr, w1_d, w2_d)
                else:
                    _ffn_full(nc, tc, g, lyr, w1_d, w2_d)

            outT = pp.tile([128, KT, BLOC], F16)
            nc.vector.tensor_copy(outT[:], g.xTl[:])
            nc.sync.dma_start(out_d[:], outT[:])

    _BUILD_CACHE[depth] = nc
    return nc


def _to8(w):
    return np.clip(w * WS, -240.0, 240.0).astype(ml_dtypes.float8_e4m3)


def kernel(**inputs):
    depth = _DEPTH
    te = np.asarray(inputs['text_encodings'], np.float32)
    tex = np.asarray(inputs['text_embed'], np.float32)
    tt = np.asarray(inputs['time_emb_table'], np.float32)
    lq = np.asarray(inputs['learned_query'], np.float32)
    rbt = np.asarray(inputs['rel_bias_table'], np.float32)
    ag = np.asarray(inputs['attn_gamma'], np.float32)
    Wq = np.asarray(inputs['Wq'], np.float32)
    Wkv = np.asarray(inputs['Wkv'], np.float32)
    Wout = np.asarray(inputs['Wout'], np.float32)
    nkv = np.asarray(inputs['null_kv'], np.float32)
    fg = np.asarray(inputs['ff_gamma'], np.float32)
    W1 = np.asarray(inputs['Wff1'], np.float32)
    W2 = np.asarray(inputs['Wff2'], np.float32)
    ts = np.asarray(inputs['diffusion_timesteps'])
    mask = np.asarray(inputs['mask'])

    time_embed = tt[ts]
    tokens = np.concatenate(
        [te, tex[:, None, :], time_embed[:, None, :],
         np.broadcast_to(lq, (B, 1, DIM))], axis=1).astype(np.float32)

    # gamma folds into the norm-consuming weights; the rmsnorm sqrt(DIM)
    # factor lives in `inv` on-chip; DH^-0.5 lives in the q descale.
    wq_eff = ag[:, :, None] * Wq
    wkv_eff = ag[:, :, None] * Wkv
    wkk_eff = np.concatenate([wkv_eff[:, :, :DH], wkv_eff[:, :, :DH]], axis=2)
    wv_eff = wkv_eff[:, :, DH:]
    w1_eff = fg[:, :, None] * W1

    def pack8(w):
        # [depth, DIM, N] -> [depth, 128, 3, 2, N] fp8 (DoubleRow pairs)
        d, K, N = w.shape
        return np.ascontiguousarray(
            _to8(w).reshape(d, NPAIR, 2, 128, N).transpose(0, 3, 1, 2, 4))

    # Wout rows permuted to the AV-output layout: aoT[p, hh, t] holds head
    # h = 2*hh + (p>=64), dim d = p%64  ->  Wout row (2*(2j+s)+(p>=64))*64+p%64
    pidx = np.arange(128)
    par = (pidx >= 64).astype(np.int64)
    dd = pidx % 64
    inner_idx = np.empty((128, NPAIR, 2), np.int64)
    for j in range(NPAIR):
        for s in range(2):
            inner_idx[:, j, s] = (2 * (2 * j + s) + par) * 64 + dd
    wo8 = _to8(Wout[:depth])           # [d, 768, 768] fp8
    woP = np.ascontiguousarray(wo8[:, inner_idx, :])  # [d, 128, 3, 2, 768]

    # scores multiplicative bias exp(relpos + causal): [81, 2(par), 6(hh), 80(i)]
    bias = _host_bias(rbt)
    causal = (np.arange(NKEY)[None, :] > np.arange(NSEQ)[:, None] + 1)
    bias = bias + np.where(causal, NEG, 0.0)[None]
    bt = np.zeros((NKEY, 2, KT, NSEQ), np.float32)
    for h in range(HEADS):
        bt[:, h % 2, h // 2, :] = bias[h].T
    with np.errstate(under='ignore'):
        expB = np.ascontiguousarray(np.exp(bt.reshape(NKEY, 960))).astype(np.float16)

    # per-batch additive key-mask rows [B, 81] (applied inside exp)
    m = np.zeros((B, NKEY), np.float32)
    not_all = mask.any(axis=-1)
    m[:, 1:L + 1] = np.where(mask, 0.0, NEG)
    m[:, L + 1] = np.where(not_all, 0.0, NEG)

    w1f = w1_eff[:depth].astype(np.float16)  # [d, DIM, 2*FF]
    d = w1f.shape[0]
    # [d, kt, p, ag, mp, n] -> [d, 128(p), 24(mp), 2(ag), 6(kt), 128(n)]
    w1p = np.ascontiguousarray(
        w1f.reshape(d, KT, 128, 2, FKT, 128).transpose(0, 2, 4, 3, 1, 5))
    # [d, FF, DIM] -> [d, 128(p), 6(m), 24(fk), 128(n)]
    w2p = np.ascontiguousarray(
        W2[:depth].astype(np.float16).reshape(d, FKT, 128, KT, 128)
        .transpose(0, 2, 3, 1, 4))

    nc = _build(depth)
    shared = {
        "wq": pack8(wq_eff[:depth]),
        "wkk": pack8(wkk_eff[:depth]),
        "wv": pack8(wv_eff[:depth]),
        "wo": woP,
        "w1": w1p,
        "w2": w2p,
        "nk2": np.ascontiguousarray(
            np.concatenate([nkv[:depth, 0], nkv[:depth, 0]], axis=1)
            .reshape(depth, 128, 1)),
        "nv": np.ascontiguousarray(nkv[:depth, 1].reshape(depth, DH, 1)),
        "expB": expB,
    }
    in_maps = []
    for c in range(NCORES):
        bsl = slice(c * BLOC, (c + 1) * BLOC)
        im = dict(shared)
        xTc = tokens[bsl].reshape(TLOC, DIM).T  # [DIM, TLOC]
        im["xT"] = np.ascontiguousarray(
            xTc.reshape(KT, 128, TLOC).transpose(1, 0, 2)).astype(np.float16)
        im["maskT"] = np.ascontiguousarray(m[bsl].T)
        in_maps.append(im)

    res = run_bass_kernel_spmd(nc, in_maps, core_ids=list(range(NCORES)),
                               trace=bool(int(os.environ.get('KERNEL_TRACE', '0'))))
    outs = []
    for c in range(NCORES):
        o = res.results[c]["out"]  # [128(p), KT, BLOC] f16
        outs.append(np.transpose(o, (2, 1, 0)).reshape(BLOC, DIM).astype(np.float32))
    kernel.last_results = res
    return np.concatenate(outs, axis=0)
